# revision 1
# baseline (speedup 1.0000x reference)
"""Self-contained Trainium2 Bass kernel for nn_Deform (bilinear grid sample).

kernel(source, motions) -> (16, 11, 256, 256, 3) float32
Shards the 176 (bs*K) grids across 8 NeuronCores (22 grids each); the
source image is replicated. See build_kernel docstring for the device
algorithm.
"""

"""Bilinear grid-sample (Deform) Bass kernel for TRN2.

Strategy:
- 12 fp16 "pair planes" per 16-partition group: plane j = (c, dy, v) holds
  pairs (img[y+dy, 2q+v], img[y+dy, 2q+v+1]) at pair index i = y*128 + q,
  zero outside the image. All 8 Q7 groups hold identical plane sets.
- One ap_gather index per pixel: i0 = clamp(yn,0,255)*128 + clamp(xw,0,255)>>1.
  Gather returns, for pixel slot m of group g, the 12 planes' pairs.
- Per-pixel blend coefficients ACE[dy,vv,e] = wy_dy*masky_dy * wx_e*maskx_e *
  (parity==vv), computed on DVE in "L1" layout, moved to gather-aligned rows
  by strided SBUF->SBUF DMAs, broadcast to the 12 plane rows by a constant
  selector matmul on PE.
- out_c = sum_{dy,vv,e} ACE * G[(c,dy,vv), m, e]; reduced via PE transpose +
  DVE strided segment reduce; one DMA per 512-pixel chunk to HBM.

Pixel id n (per core) = call*(8*N) + g*N + m, m = f*16 + r.
"""

import numpy as np

import concourse.bass as bass
import concourse.mybir as mybir

F32 = mybir.dt.float32
F16 = mybir.dt.float16
I16 = mybir.dt.int16

H = W = 256
C = 3
NE = 32768          # pairs per plane (= 256*128)
NPLANE = 12         # (c, dy, v)


def plane_id(c, dy, v):
    return c * 4 + dy * 2 + v


def make_selector(e):
    """S[k, mcol] fp16: CW_e[16g + j] <- QR row 16g + t(j), t = dy*4 + v*2 + e."""
    S = np.zeros((128, 128), np.float16)
    for g in range(8):
        for j in range(NPLANE):
            c, rem = divmod(j, 4)
            dy, v = divmod(rem, 2)
            t = dy * 4 + v * 2 + e
            S[16 * g + t, 16 * g + j] = 1.0
    return S


def make_identity128():
    return np.eye(128, dtype=np.float16)


def build_kernel(nc, n_pix, N=4096, dbg=False, skip_gather=False, only_gather=False):
    """Emit the full kernel into `nc`. n_pix must be a multiple of 8*N."""
    from concourse.tile import TileContext

    CALL_PIX = 8 * N
    assert n_pix % CALL_PIX == 0
    n_calls = n_pix // CALL_PIX
    F = N // 16          # slots per partition-lane
    NQ = 8               # quantities (coefficients) per pixel
    CH = 512             # blend chunk (pixels-slots per group)
    n_chunks = N // CH

    src = nc.dram_tensor("source", [1, H, W, C], F32, kind="ExternalInput")
    mot = nc.dram_tensor("motions", [n_pix, 2], F32, kind="ExternalInput")
    sel0_d = nc.dram_tensor("sel0", [128, 128], F16, kind="ExternalInput")
    sel1_d = nc.dram_tensor("sel1", [128, 128], F16, kind="ExternalInput")
    ident_d = nc.dram_tensor("ident", [128, 128], F16, kind="ExternalInput")
    out = nc.dram_tensor("out", [n_pix, C], F32, kind="ExternalOutput")

    with TileContext(nc) as tc:
        with (
            tc.tile_pool(name="pln", bufs=1) as plnp,
            tc.tile_pool(name="const", bufs=1) as constp,
            tc.tile_pool(name="work", bufs=1) as workp,
            tc.tile_pool(name="pipe", bufs=2) as pipep,
            tc.tile_pool(name="psum", bufs=2, space="PSUM") as psump,
            tc.tile_pool(name="dram", bufs=1, space="DRAM") as dramp,
        ):
            # ---- constants ----
            sel0 = constp.tile([128, 128], F16)
            sel1 = constp.tile([128, 128], F16)
            ident = constp.tile([128, 128], F16)
            nc.sync.dma_start(sel0[:], sel0_d[:])
            nc.sync.dma_start(sel1[:], sel1_d[:])
            nc.sync.dma_start(ident[:], ident_d[:])

            # ---- plane table: [128, NE, 2] fp16 ----
            # stage fp16 channel images (+pad) in DRAM, then planes are
            # contiguous shifted reads (edge garbage is weight-masked).
            PAD = 512
            ch_d = dramp.tile([3, H * W + PAD], F16)
            s0 = pipep.tile([128, 512, 3], F32, tag="gth")
            s16 = pipep.tile([128, 3, 512], F16, tag="qr16")
            zz = pipep.tile([128, 4], F16, tag="idx")
            nc.sync.dma_start(
                s0[:], src[0].rearrange("h w c -> (h w) c").rearrange(
                    "(p k) c -> p k c", p=128))
            nc.vector.tensor_copy(s16[:], s0[:].rearrange("p k c -> p c k"))
            nc.vector.memset(zz[:], 0.0)
            for c in range(C):
                nc.sync.dma_start(
                    ch_d[c, :H * W].rearrange("(p k) -> p k", p=128),
                    s16[:, c, :])
            nc.sync.dma_start(
                ch_d[:, H * W:].rearrange("c (p q) -> p c q", p=128),
                zz[:128, :].rearrange("p (c q) -> p c q", c=1).to_broadcast([128, 3, 4]))
            pln = plnp.tile([128, NE, 2], F16)
            nc.vector.memset(pln[:, :NE // 2, :], 0.0)
            nc.vector.memset(pln[:, NE // 2:, :], 0.0)
            flat = pln[:].rearrange("p n d -> p (n d)")  # [128, 65536]
            for g in range(8):
                for c in range(C):
                    for dy in range(2):
                        for v in range(2):
                            p = 16 * g + plane_id(c, dy, v)
                            off = dy * W + v
                            nc.sync.dma_start(
                                flat[p:p + 1, :],
                                ch_d[c, off:off + H * W],
                            )

            # zero qr16 pool slots once: rows 16g+8..16 are never written
            for _ in range(2):
                qz = pipep.tile([128, N], F16, tag="qr16")
                nc.vector.memset(qz[:], 0.0)

            # ---- main loop ----
            for call in range(n_calls):
                base = call * CALL_PIX

                # motions tile (wrap layout): partition 16g+r <- pixel base + g*N + f*16 + r
                motv = pipep.tile([128, F, 2], F32, tag="motv")
                # motions tile (r-major): partition 16g+r <- pixels [g*N + r*F, +F)
                motq = pipep.tile([128, F, 2], F32, tag="motq")
                for g in range(8):
                    nc.sync.dma_start(
                        motv[16 * g:16 * g + 16, :, :],
                        mot[base + g * N: base + (g + 1) * N, :].rearrange(
                            "(f r) d -> r f d", r=16),
                    )
                    nc.sync.dma_start(
                        motq[16 * g:16 * g + 16, :, :],
                        mot[base + g * N: base + (g + 1) * N, :].rearrange(
                            "(r f) d -> r f d", r=16),
                    )
                gx = motv[:, :, 0]
                gy = motv[:, :, 1]
                qx = motq[:, :, 0]
                qy = motq[:, :, 1]

                # ---- phase A: per-pixel quantities (f32, L1 layout) ----
                q8 = workp.tile([128, NQ, F], F16, tag="q8")
                idx = pipep.tile([128, F], I16, tag="idx")
                t1 = workp.tile([128, F], F32, tag="t1")
                t2 = workp.tile([128, F], F32, tag="t2")
                txw = workp.tile([128, F], F32, tag="txw")
                tyw = workp.tile([128, F], F32, tag="tyw")
                fx = workp.tile([128, F], F32, tag="fx")
                fy = workp.tile([128, F], F32, tag="fy")
                b0 = workp.tile([128, F], F32, tag="b0")
                b1 = workp.tile([128, F], F32, tag="b1")
                a0 = workp.tile([128, F], F32, tag="a0")
                a1 = workp.tile([128, F], F32, tag="a1")
                vpar = tyw   # alias: tyw dead before parity computation
                i0f = fx     # alias: fx dead before i0 computation

                AL = mybir.AluOpType
                # ---- idx path (wrap layout) ----
                nc.vector.tensor_scalar(t1[:], gx, 128.0, 143.5, AL.mult, AL.add)
                nc.vector.tensor_scalar(t2[:], t1[:], 12582912.0, 12582912.0, AL.add, AL.subtract)
                nc.vector.tensor_tensor(fy[:], t2[:], t1[:], AL.is_gt)
                nc.vector.tensor_tensor(txw[:], t2[:], fy[:], AL.subtract)
                nc.vector.tensor_scalar(t1[:], gy, 128.0, 143.5, AL.mult, AL.add)
                nc.vector.tensor_scalar(t2[:], t1[:], 12582912.0, 12582912.0, AL.add, AL.subtract)
                nc.vector.tensor_tensor(fy[:], t2[:], t1[:], AL.is_gt)
                nc.vector.tensor_tensor(tyw[:], t2[:], fy[:], AL.subtract)
                nc.vector.tensor_scalar(t1[:], txw[:], 16.0, 271.0, AL.max, AL.min)
                nc.vector.tensor_scalar(t1[:], t1[:], 0.5, -8.0, AL.mult, AL.add)
                nc.vector.tensor_scalar(fy[:], t1[:], 12582912.0, 12582912.0, AL.add, AL.subtract)
                nc.vector.tensor_tensor(b0[:], fy[:], t1[:], AL.is_gt)
                nc.vector.tensor_tensor(t2[:], fy[:], b0[:], AL.subtract)
                nc.vector.tensor_scalar(t1[:], tyw[:], 16.0, 271.0, AL.max, AL.min)
                nc.vector.tensor_scalar(t2[:], t2[:], -2048.0, None, AL.add)
                nc.vector.scalar_tensor_tensor(
                    i0f[:], t1[:], 128.0, t2[:], AL.mult, AL.add)
                nc.vector.tensor_copy(idx[:], i0f[:])

                # ---- coefficient path (r-major layout) ----
                nc.vector.tensor_scalar(t1[:], qx, 128.0, 143.5, AL.mult, AL.add)
                nc.vector.tensor_scalar(t2[:], t1[:], 12582912.0, 12582912.0, AL.add, AL.subtract)
                nc.vector.tensor_tensor(fx[:], t2[:], t1[:], AL.is_gt)
                nc.vector.tensor_tensor(txw[:], t2[:], fx[:], AL.subtract)
                nc.vector.tensor_tensor(fx[:], t1[:], txw[:], AL.subtract)
                nc.vector.tensor_scalar(t1[:], qy, 128.0, 143.5, AL.mult, AL.add)
                nc.vector.tensor_scalar(t2[:], t1[:], 12582912.0, 12582912.0, AL.add, AL.subtract)
                nc.vector.tensor_tensor(fy[:], t2[:], t1[:], AL.is_gt)
                nc.vector.tensor_tensor(tyw[:], t2[:], fy[:], AL.subtract)
                nc.vector.tensor_tensor(fy[:], t1[:], tyw[:], AL.subtract)
                # x masks -> B0/B1
                nc.vector.tensor_scalar(t1[:], txw[:], 15.0, None, AL.is_gt)
                nc.vector.tensor_scalar(t2[:], txw[:], 272.0, None, AL.is_lt)
                nc.vector.tensor_tensor(b0[:], t1[:], t2[:], AL.mult)
                nc.vector.tensor_scalar(t1[:], txw[:], 14.0, None, AL.is_gt)
                nc.vector.tensor_scalar(t2[:], txw[:], 271.0, None, AL.is_lt)
                nc.vector.tensor_tensor(b1[:], t1[:], t2[:], AL.mult)
                nc.vector.tensor_scalar(t1[:], fx[:], -1.0, 1.0, AL.mult, AL.add)
                nc.vector.tensor_tensor(b0[:], b0[:], t1[:], AL.mult)
                nc.vector.tensor_tensor(b1[:], b1[:], fx[:], AL.mult)
                # xw == -1: e-corner lands in pair elem 0 -> swap coefficients
                nc.vector.tensor_scalar(t1[:], txw[:], 15.0, None, AL.is_equal)
                nc.vector.tensor_tensor(t2[:], t1[:], b1[:], AL.mult)
                nc.vector.tensor_tensor(b0[:], b0[:], t2[:], AL.add)
                nc.vector.tensor_scalar(t1[:], t1[:], -1.0, 1.0, AL.mult, AL.add)
                nc.vector.tensor_tensor(b1[:], b1[:], t1[:], AL.mult)
                # y masks -> A0/A1
                nc.vector.tensor_scalar(t1[:], tyw[:], 15.0, None, AL.is_gt)
                nc.vector.tensor_scalar(t2[:], tyw[:], 272.0, None, AL.is_lt)
                nc.vector.tensor_tensor(a0[:], t1[:], t2[:], AL.mult)
                nc.vector.tensor_scalar(t1[:], tyw[:], 14.0, None, AL.is_gt)
                nc.vector.tensor_scalar(t2[:], tyw[:], 271.0, None, AL.is_lt)
                nc.vector.tensor_tensor(a1[:], t1[:], t2[:], AL.mult)
                nc.vector.tensor_scalar(t1[:], fy[:], -1.0, 1.0, AL.mult, AL.add)
                nc.vector.tensor_tensor(a0[:], a0[:], t1[:], AL.mult)
                nc.vector.tensor_tensor(a1[:], a1[:], fy[:], AL.mult)
                # yn == -1: s-corner lands in dy=0 plane -> swap coefficients
                nc.vector.tensor_scalar(t1[:], tyw[:], 15.0, None, AL.is_equal)
                nc.vector.tensor_tensor(t2[:], t1[:], a1[:], AL.mult)
                nc.vector.tensor_tensor(a0[:], a0[:], t2[:], AL.add)
                nc.vector.tensor_scalar(t1[:], t1[:], -1.0, 1.0, AL.mult, AL.add)
                nc.vector.tensor_tensor(a1[:], a1[:], t1[:], AL.mult)
                # parity v
                nc.vector.tensor_scalar(t1[:], txw[:], 16.0, 271.0, AL.max, AL.min)
                nc.vector.tensor_scalar(t1[:], t1[:], 0.5, -8.0, AL.mult, AL.add)
                nc.vector.tensor_scalar(t2[:], t1[:], 12582912.0, 12582912.0, AL.add, AL.subtract)
                nc.vector.tensor_tensor(fx[:], t2[:], t1[:], AL.is_gt)
                nc.vector.tensor_tensor(vpar[:], t2[:], fx[:], AL.subtract)
                nc.vector.tensor_tensor(vpar[:], t1[:], vpar[:], AL.subtract)
                nc.vector.tensor_scalar(vpar[:], vpar[:], 2.0, None, AL.mult)
                # ACE quantities t = dy*4 + vv*2 + e
                nc.vector.tensor_scalar(t1[:], vpar[:], -1.0, 1.0, AL.mult, AL.add)
                for dy, at in ((0, a0), (1, a1)):
                    for e, bt in ((0, b0), (1, b1)):
                        nc.vector.tensor_tensor(t2[:], at[:], bt[:], AL.mult)
                        nc.vector.tensor_tensor(
                            q8[:, dy * 4 + 0 * 2 + e, :], t2[:], t1[:], AL.mult)
                        nc.vector.tensor_tensor(
                            q8[:, dy * 4 + 1 * 2 + e, :], t2[:], vpar[:], AL.mult)

                # ---- phase C: unwrap DMAs -> QR [128, N] f32 rows 16g+t ----
                qr16 = pipep.tile([128, N], F16, tag="qr16")
                qstage = dramp.tile([8, NQ, N], F16, tag="qstage")
                for g in range(8):
                    for t in range(NQ):
                        nc.sync.dma_start(
                            qstage[g, t, :].rearrange("(r f) -> r f", r=16),
                            q8[16 * g:16 * g + 16, t, :],
                        )
                    nc.sync.dma_start(
                        qr16[16 * g:16 * g + NQ, :],
                        qstage[g],
                    )

                # ---- phase D: gather ----
                gth = pipep.tile([128, N, 2], F16, tag="gth")
                if skip_gather:
                    pass
                else:
                        nc.gpsimd.ap_gather(
                        out_ap=gth[:],
                        in_ap=pln[:],
                        idxs_ap=idx[:],
                        channels=128,
                        num_elems=NE,
                        d=2,
                        num_idxs=N,
                    )
                if only_gather:
                    continue

                # ---- phase E/F: blend per chunk ----
                og = workp.tile([128, n_chunks, 4, 24], F32, tag="og")
                for ch in range(n_chunks):
                    sl = slice(ch * CH, (ch + 1) * CH)
                    cw0 = psump.tile([128, CH], F32, tag="cw0")
                    cw1 = psump.tile([128, CH], F32, tag="cw1")
                    nc.tensor.matmul(cw0[:], sel0[:], qr16[:, sl], start=True, stop=True)
                    nc.tensor.matmul(cw1[:], sel1[:], qr16[:, sl], start=True, stop=True)
                    g0 = workp.tile([128, CH], F32, tag="g0")
                    g1 = workp.tile([128, CH], F32, tag="g1")
                    nc.vector.tensor_tensor(g0[:], gth[:, sl, 0], cw0[:], AL.mult)
                    nc.vector.tensor_tensor(g1[:], gth[:, sl, 1], cw1[:], AL.mult)
                    rr = workp.tile([128, CH], F16, tag="rr")
                    nc.vector.tensor_tensor(rr[:], g0[:], g1[:], AL.add)
                    # transpose rr -> psum [128, 4*128]
                    rt = psump.tile([128, 512], F16, tag="rt")
                    for B in range(4):
                        nc.tensor.transpose(
                            rt[:, B * 128:(B + 1) * 128],
                            rr[:, B * 128:(B + 1) * 128],
                            ident[:],
                        )
                    # reduce segments of 4 (dy,vv) within each (g, c)
                    for B in range(4):
                        nc.vector.tensor_reduce(
                            og[:, ch, B, :].rearrange("p (g c) -> p g c", g=8),
                            rt[:, B * 128:(B + 1) * 128].rearrange(
                                "p (g j) -> p g j", g=8)[:, :, 0:12].rearrange(
                                "p g (c s) -> p g c s", c=3),
                            op=AL.add, axis=mybir.AxisListType.X,
                        )
                # out DMA: one per chunk-call... do per chunk here
                for ch in range(n_chunks):
                    for B in range(4):
                        # pixels n = base + g*N + ch*CH + B*128 + m128, channel c
                        nc.sync.dma_start(
                            out[base:base + CALL_PIX, :].rearrange(
                                "(g ck b m) c -> m ck b g c",
                                g=8, ck=n_chunks, b=4)[:, ch, B, :, :],
                            og[:, ch, B, :].rearrange("p (g c) -> p g c", g=8),
                        )
    return nc


_COMPILED = {}


def _get_compiled(n_pix, N):
    key = (n_pix, N)
    if key not in _COMPILED:
        import concourse.bacc as bacc
        nc = bacc.Bacc("TRN2", target_bir_lowering=False, debug=False)
        build_kernel(nc, n_pix, N=N)
        nc.finalize()
        _COMPILED[key] = nc
    return _COMPILED[key]


def kernel(source, motions):
    from concourse import bass_utils

    bs, K, h, w, _ = motions.shape          # (16, 11, 256, 256, 2)
    n_cores = 8
    grids = bs * K                          # 176
    per_core = grids // n_cores             # 22
    n_pix = per_core * h * w                # 1441792
    N = 4096

    nc = _get_compiled(n_pix, N)

    mot_flat = np.ascontiguousarray(
        motions.reshape(grids, h * w, 2).reshape(n_cores, n_pix, 2))
    src = np.ascontiguousarray(source, dtype=np.float32)
    sel0 = make_selector(0)
    sel1 = make_selector(1)
    ident = make_identity128()

    in_maps = [
        {
            "source": src,
            "motions": np.ascontiguousarray(mot_flat[i]),
            "sel0": sel0,
            "sel1": sel1,
            "ident": ident,
        }
        for i in range(n_cores)
    ]
    res = bass_utils.run_bass_kernel_spmd(nc, in_maps, core_ids=list(range(n_cores)))
    out = np.stack([res.results[i]["out"] for i in range(n_cores)], axis=0)
    return out.reshape(bs, K, h, w, 3)



# revision 9
# speedup vs baseline: 1.4808x; 1.4808x over previous
"""Self-contained Trainium2 Bass kernel for nn_Deform (bilinear grid sample).

kernel(source, motions) -> (16, 11, 256, 256, 3) float32
Shards the 176 (bs*K) grids across 8 NeuronCores (22 grids each); the
source image is replicated. See build_kernel docstring for the device
algorithm.
"""

"""Bilinear grid-sample (Deform) Bass kernel for TRN2.

Strategy:
- 12 fp16 "pair planes" per 16-partition group: plane j = (c, dy, v) holds
  pairs (img[y+dy, 2q+v], img[y+dy, 2q+v+1]) at pair index i = y*128 + q,
  zero outside the image. All 8 Q7 groups hold identical plane sets.
- One ap_gather index per pixel: i0 = clamp(yn,0,255)*128 + clamp(xw,0,255)>>1.
  Gather returns, for pixel slot m of group g, the 12 planes' pairs.
- Per-pixel blend coefficients ACE[dy,vv,e] = wy_dy*masky_dy * wx_e*maskx_e *
  (parity==vv), computed on DVE in "L1" layout, moved to gather-aligned rows
  by strided SBUF->SBUF DMAs, broadcast to the 12 plane rows by a constant
  selector matmul on PE.
- out_c = sum_{dy,vv,e} ACE * G[(c,dy,vv), m, e]; reduced via PE transpose +
  DVE strided segment reduce; one DMA per 512-pixel chunk to HBM.

Pixel id n (per core) = call*(8*N) + g*N + m, m = f*16 + r.
"""

import numpy as np

import concourse.bass as bass
import concourse.mybir as mybir

F32 = mybir.dt.float32
F16 = mybir.dt.float16
I16 = mybir.dt.int16

H = W = 256
C = 3
NE = 32768          # pairs per plane (= 256*128)
NPLANE = 12         # (c, dy, v)


def plane_id(c, dy, v):
    return c * 4 + dy * 2 + v


def make_selector(e):
    """S[k, mcol] fp16: CW_e[16g + j] <- QR row 16g + t(j), t = dy*4 + v*2 + e."""
    S = np.zeros((128, 128), np.float16)
    for g in range(8):
        for j in range(NPLANE):
            c, rem = divmod(j, 4)
            dy, v = divmod(rem, 2)
            t = dy * 4 + v * 2 + e
            S[16 * g + t, 16 * g + j] = 1.0
    return S


def make_identity128():
    return np.eye(128, dtype=np.float16)


def build_kernel(nc, n_pix, N=4096, dbg=False, skip_gather=False, only_gather=False):
    """Emit the full kernel into `nc`. n_pix must be a multiple of 8*N."""
    from concourse.tile import TileContext

    CALL_PIX = 8 * N
    assert n_pix % CALL_PIX == 0
    n_calls = n_pix // CALL_PIX
    F = N // 16          # slots per partition-lane
    NQ = 8               # quantities (coefficients) per pixel
    CH = 512             # blend chunk (pixels-slots per group)
    n_chunks = N // CH

    src = nc.dram_tensor("source", [1, H, W, C], F32, kind="ExternalInput")
    # pre-permuted on host to the exact SBUF tile layouts (fat DMAs)
    mot_w = nc.dram_tensor("motions_w", [n_calls, 128, N // 16, 2], F32,
                           kind="ExternalInput")
    mot_r = nc.dram_tensor("motions_r", [n_calls, 128, N // 16, 2], F32,
                           kind="ExternalInput")
    sel0_d = nc.dram_tensor("sel0", [128, 128], F16, kind="ExternalInput")
    sel1_d = nc.dram_tensor("sel1", [128, 128], F16, kind="ExternalInput")
    ident_d = nc.dram_tensor("ident", [128, 128], F16, kind="ExternalInput")
    # device-layout output; host unscrambles
    out = nc.dram_tensor("out", [n_calls, 128, N // CH, 4, 24], F16,
                         kind="ExternalOutput")

    with TileContext(nc) as tc:
        with (
            tc.tile_pool(name="pln", bufs=1) as plnp,
            tc.tile_pool(name="const", bufs=1) as constp,
            tc.tile_pool(name="work", bufs=1) as workp,
            tc.tile_pool(name="pipe", bufs=2) as pipep,
            tc.tile_pool(name="psum", bufs=2, space="PSUM") as psump,
            tc.tile_pool(name="dram", bufs=1, space="DRAM") as dramp,
        ):
            # ---- constants ----
            sel0 = constp.tile([128, 128], F16)
            sel1 = constp.tile([128, 128], F16)
            ident = constp.tile([128, 128], F16)
            nc.sync.dma_start(sel0[:], sel0_d[:])
            nc.sync.dma_start(sel1[:], sel1_d[:])
            nc.sync.dma_start(ident[:], ident_d[:])

            # ---- plane table: [128, NE, 2] fp16 ----
            # stage fp16 channel images (+pad) in DRAM, then planes are
            # contiguous shifted reads (edge garbage is weight-masked).
            PAD = 512
            ch_d = dramp.tile([3, H * W + PAD], F16)
            s0 = pipep.tile([128, 512, 3], F32, tag="gth")
            s16 = pipep.tile([128, 3, 512], F16, tag="qr16")
            zz = pipep.tile([128, 4], F16, tag="idx")
            nc.sync.dma_start(
                s0[:], src[0].rearrange("h w c -> (h w) c").rearrange(
                    "(p k) c -> p k c", p=128))
            nc.vector.tensor_copy(s16[:], s0[:].rearrange("p k c -> p c k"))
            nc.vector.memset(zz[:], 0.0)
            for c in range(C):
                nc.sync.dma_start(
                    ch_d[c, :H * W].rearrange("(p k) -> p k", p=128),
                    s16[:, c, :])
            nc.sync.dma_start(
                ch_d[:, H * W:].rearrange("c (p q) -> p c q", p=128),
                zz[:128, :].rearrange("p (c q) -> p c q", c=1).to_broadcast([128, 3, 4]))
            pln = plnp.tile([128, NE, 2], F16)
            nc.vector.memset(pln[:, :NE // 2, :], 0.0)
            nc.vector.memset(pln[:, NE // 2:, :], 0.0)
            flat = pln[:].rearrange("p n d -> p (n d)")  # [128, 65536]
            for g in range(8):
                for c in range(C):
                    for dy in range(2):
                        for v in range(2):
                            p = 16 * g + plane_id(c, dy, v)
                            off = dy * W + v
                            nc.sync.dma_start(
                                flat[p:p + 1, :],
                                ch_d[c, off:off + H * W],
                            )

            # zero qr16 pool slots once: rows 16g+8..16 are never written
            for _ in range(2):
                qz = pipep.tile([128, N], F16, tag="qr16")
                nc.vector.memset(qz[:], 0.0)

            # ---- main loop ----
            for call in range(n_calls):
                # motions tile (wrap layout): partition 16g+r <- pixel base + g*N + f*16 + r
                motv = pipep.tile([128, F, 2], F32, tag="motv")
                # motions tile (r-major): partition 16g+r <- pixels [g*N + r*F, +F)
                motq = pipep.tile([128, F, 2], F32, tag="motq")
                nc.sync.dma_start(motv[:], mot_w[call])
                nc.sync.dma_start(motq[:], mot_r[call])
                gx = motv[:, :, 0]
                gy = motv[:, :, 1]
                qx = motq[:, :, 0]
                qy = motq[:, :, 1]

                # ---- phase A: per-pixel quantities (f32, L1 layout) ----
                q8 = workp.tile([128, NQ, F], F16, tag="q8")
                idx = pipep.tile([128, F], I16, tag="idx")
                t1 = workp.tile([128, F], F32, tag="t1")
                t2 = workp.tile([128, F], F32, tag="t2")
                txw = workp.tile([128, F], F32, tag="txw")
                tyw = workp.tile([128, F], F32, tag="tyw")
                fx = workp.tile([128, F], F32, tag="fx")
                fy = workp.tile([128, F], F32, tag="fy")
                b0 = workp.tile([128, F], F32, tag="b0")
                b1 = workp.tile([128, F], F32, tag="b1")
                a0 = workp.tile([128, F], F32, tag="a0")
                a1 = workp.tile([128, F], F32, tag="a1")
                vpar = tyw   # alias: tyw dead before parity computation
                i0f = fx     # alias: fx dead before i0 computation

                AL = mybir.AluOpType
                # ---- idx path (wrap layout) ----
                nc.vector.tensor_scalar(t1[:], gx, 128.0, 143.5, AL.mult, AL.add)
                nc.vector.tensor_scalar(t2[:], t1[:], 12582912.0, 12582912.0, AL.add, AL.subtract)
                nc.vector.tensor_tensor(fy[:], t2[:], t1[:], AL.is_gt)
                nc.vector.tensor_tensor(txw[:], t2[:], fy[:], AL.subtract)
                nc.vector.tensor_scalar(t1[:], gy, 128.0, 143.5, AL.mult, AL.add)
                nc.vector.tensor_scalar(t2[:], t1[:], 12582912.0, 12582912.0, AL.add, AL.subtract)
                nc.vector.tensor_tensor(fy[:], t2[:], t1[:], AL.is_gt)
                nc.vector.tensor_tensor(tyw[:], t2[:], fy[:], AL.subtract)
                nc.vector.tensor_scalar(t1[:], txw[:], 16.0, 271.0, AL.max, AL.min)
                nc.vector.tensor_scalar(t1[:], t1[:], 0.5, -8.0, AL.mult, AL.add)
                nc.vector.tensor_scalar(fy[:], t1[:], 12582912.0, 12582912.0, AL.add, AL.subtract)
                nc.vector.tensor_tensor(b0[:], fy[:], t1[:], AL.is_gt)
                nc.vector.tensor_tensor(t2[:], fy[:], b0[:], AL.subtract)
                nc.vector.tensor_scalar(t1[:], tyw[:], 16.0, 271.0, AL.max, AL.min)
                nc.vector.tensor_scalar(t2[:], t2[:], -2048.0, None, AL.add)
                nc.vector.scalar_tensor_tensor(
                    i0f[:], t1[:], 128.0, t2[:], AL.mult, AL.add)
                nc.vector.tensor_copy(idx[:], i0f[:])

                # ---- coefficient path (r-major layout) ----
                nc.vector.tensor_scalar(t1[:], qx, 128.0, 143.5, AL.mult, AL.add)
                nc.vector.tensor_scalar(t2[:], t1[:], 12582912.0, 12582912.0, AL.add, AL.subtract)
                nc.vector.tensor_tensor(fx[:], t2[:], t1[:], AL.is_gt)
                nc.vector.tensor_tensor(txw[:], t2[:], fx[:], AL.subtract)
                nc.vector.tensor_tensor(fx[:], t1[:], txw[:], AL.subtract)
                nc.vector.tensor_scalar(t1[:], qy, 128.0, 143.5, AL.mult, AL.add)
                nc.vector.tensor_scalar(t2[:], t1[:], 12582912.0, 12582912.0, AL.add, AL.subtract)
                nc.vector.tensor_tensor(fy[:], t2[:], t1[:], AL.is_gt)
                nc.vector.tensor_tensor(tyw[:], t2[:], fy[:], AL.subtract)
                nc.vector.tensor_tensor(fy[:], t1[:], tyw[:], AL.subtract)
                # x masks -> B0/B1
                nc.vector.tensor_scalar(t1[:], txw[:], 15.0, None, AL.is_gt)
                nc.vector.tensor_scalar(t2[:], txw[:], 272.0, None, AL.is_lt)
                nc.vector.tensor_tensor(b0[:], t1[:], t2[:], AL.mult)
                nc.vector.tensor_scalar(t1[:], txw[:], 14.0, None, AL.is_gt)
                nc.vector.tensor_scalar(t2[:], txw[:], 271.0, None, AL.is_lt)
                nc.vector.tensor_tensor(b1[:], t1[:], t2[:], AL.mult)
                nc.vector.tensor_scalar(t1[:], fx[:], -1.0, 1.0, AL.mult, AL.add)
                nc.vector.tensor_tensor(b0[:], b0[:], t1[:], AL.mult)
                nc.vector.tensor_tensor(b1[:], b1[:], fx[:], AL.mult)
                # xw == -1: e-corner lands in pair elem 0 -> swap coefficients
                nc.vector.tensor_scalar(t1[:], txw[:], 15.0, None, AL.is_equal)
                nc.vector.tensor_tensor(t2[:], t1[:], b1[:], AL.mult)
                nc.vector.tensor_tensor(b0[:], b0[:], t2[:], AL.add)
                nc.vector.tensor_scalar(t1[:], t1[:], -1.0, 1.0, AL.mult, AL.add)
                nc.vector.tensor_tensor(b1[:], b1[:], t1[:], AL.mult)
                # y masks -> A0/A1
                nc.vector.tensor_scalar(t1[:], tyw[:], 15.0, None, AL.is_gt)
                nc.vector.tensor_scalar(t2[:], tyw[:], 272.0, None, AL.is_lt)
                nc.vector.tensor_tensor(a0[:], t1[:], t2[:], AL.mult)
                nc.vector.tensor_scalar(t1[:], tyw[:], 14.0, None, AL.is_gt)
                nc.vector.tensor_scalar(t2[:], tyw[:], 271.0, None, AL.is_lt)
                nc.vector.tensor_tensor(a1[:], t1[:], t2[:], AL.mult)
                nc.vector.tensor_scalar(t1[:], fy[:], -1.0, 1.0, AL.mult, AL.add)
                nc.vector.tensor_tensor(a0[:], a0[:], t1[:], AL.mult)
                nc.vector.tensor_tensor(a1[:], a1[:], fy[:], AL.mult)
                # yn == -1: s-corner lands in dy=0 plane -> swap coefficients
                nc.vector.tensor_scalar(t1[:], tyw[:], 15.0, None, AL.is_equal)
                nc.vector.tensor_tensor(t2[:], t1[:], a1[:], AL.mult)
                nc.vector.tensor_tensor(a0[:], a0[:], t2[:], AL.add)
                nc.vector.tensor_scalar(t1[:], t1[:], -1.0, 1.0, AL.mult, AL.add)
                nc.vector.tensor_tensor(a1[:], a1[:], t1[:], AL.mult)
                # parity v
                nc.vector.tensor_scalar(t1[:], txw[:], 16.0, 271.0, AL.max, AL.min)
                nc.vector.tensor_scalar(t1[:], t1[:], 0.5, -8.0, AL.mult, AL.add)
                nc.vector.tensor_scalar(t2[:], t1[:], 12582912.0, 12582912.0, AL.add, AL.subtract)
                nc.vector.tensor_tensor(fx[:], t2[:], t1[:], AL.is_gt)
                nc.vector.tensor_tensor(vpar[:], t2[:], fx[:], AL.subtract)
                nc.vector.tensor_tensor(vpar[:], t1[:], vpar[:], AL.subtract)
                nc.vector.tensor_scalar(vpar[:], vpar[:], 2.0, None, AL.mult)
                # ACE quantities t = dy*4 + vv*2 + e
                nc.vector.tensor_scalar(t1[:], vpar[:], -1.0, 1.0, AL.mult, AL.add)
                for dy, at in ((0, a0), (1, a1)):
                    for e, bt in ((0, b0), (1, b1)):
                        nc.vector.tensor_tensor(t2[:], at[:], bt[:], AL.mult)
                        nc.vector.tensor_tensor(
                            q8[:, dy * 4 + 0 * 2 + e, :], t2[:], t1[:], AL.mult)
                        nc.vector.tensor_tensor(
                            q8[:, dy * 4 + 1 * 2 + e, :], t2[:], vpar[:], AL.mult)

                # ---- phase C: unwrap DMAs -> QR [128, N] f32 rows 16g+t ----
                qr16 = pipep.tile([128, N], F16, tag="qr16")
                qstage = dramp.tile([8, NQ, N], F16, tag="qstage")
                for g in range(8):
                    for t in range(NQ):
                        nc.sync.dma_start(
                            qstage[g, t, :].rearrange("(r f) -> r f", r=16),
                            q8[16 * g:16 * g + 16, t, :],
                        )
                    nc.sync.dma_start(
                        qr16[16 * g:16 * g + NQ, :],
                        qstage[g],
                    )

                # ---- phase D: gather ----
                gth = pipep.tile([128, N, 2], F16, tag="gth")
                if skip_gather:
                    pass
                else:
                        nc.gpsimd.ap_gather(
                        out_ap=gth[:],
                        in_ap=pln[:],
                        idxs_ap=idx[:],
                        channels=128,
                        num_elems=NE,
                        d=2,
                        num_idxs=N,
                    )
                if only_gather:
                    continue

                # ---- phase E/F: blend per chunk ----
                og = pipep.tile([128, n_chunks, 4, 24], F16, tag="og")
                for ch in range(n_chunks):
                    sl = slice(ch * CH, (ch + 1) * CH)
                    cw0 = psump.tile([128, CH], F32, tag="cw0")
                    cw1 = psump.tile([128, CH], F32, tag="cw1")
                    nc.tensor.matmul(cw0[:], sel0[:], qr16[:, sl], start=True, stop=True)
                    nc.tensor.matmul(cw1[:], sel1[:], qr16[:, sl], start=True, stop=True)
                    g0 = workp.tile([128, CH], F32, tag="g0")
                    g1 = workp.tile([128, CH], F32, tag="g1")
                    nc.vector.tensor_tensor(g0[:], gth[:, sl, 0], cw0[:], AL.mult)
                    nc.vector.tensor_tensor(g1[:], gth[:, sl, 1], cw1[:], AL.mult)
                    rr = workp.tile([128, CH], F16, tag="rr")
                    nc.vector.tensor_tensor(rr[:], g0[:], g1[:], AL.add)
                    # transpose rr -> psum [128, 4*128]
                    rt = psump.tile([128, 512], F16, tag="rt")
                    for B in range(4):
                        nc.tensor.transpose(
                            rt[:, B * 128:(B + 1) * 128],
                            rr[:, B * 128:(B + 1) * 128],
                            ident[:],
                        )
                    # reduce segments of 4 (dy,vv) within each (g, c)
                    with nc.allow_low_precision(reason="f16 4-way sums, |out|<8"):
                        for B in range(4):
                            nc.vector.tensor_reduce(
                                og[:, ch, B, :].rearrange("p (g c) -> p g c", g=8),
                                rt[:, B * 128:(B + 1) * 128].rearrange(
                                    "p (g j) -> p g j", g=8)[:, :, 0:12].rearrange(
                                    "p g (c s) -> p g c s", c=3),
                                op=AL.add, axis=mybir.AxisListType.X,
                            )
                # out DMA: device-layout, one fat contiguous DMA per call
                nc.sync.dma_start(out[call], og[:])
    return nc


_COMPILED = {}


def _get_compiled(n_pix, N):
    key = (n_pix, N)
    if key not in _COMPILED:
        import concourse.bacc as bacc
        nc = bacc.Bacc("TRN2", target_bir_lowering=False, debug=False)
        build_kernel(nc, n_pix, N=N)
        nc.finalize()
        _COMPILED[key] = nc
    return _COMPILED[key]


def prepare_in_maps(source, motions, n_cores=8, N=4096):
    """Host-side permutation into the exact device tile layouts."""
    bs, K, h, w, _ = motions.shape          # (16, 11, 256, 256, 2)
    grids = bs * K                          # 176
    n_pix = (grids // n_cores) * h * w      # 1441792 per core
    n_calls = n_pix // (8 * N)              # 44
    F = N // 16                             # 256

    mot_flat = motions.reshape(n_cores, n_pix, 2)
    src = np.ascontiguousarray(source, dtype=np.float32)
    sel0 = make_selector(0)
    sel1 = make_selector(1)
    ident = make_identity128()

    in_maps = []
    for i in range(n_cores):
        m = mot_flat[i]
        # wrap layout: [call, 16g+r, f, d] <- pixel call*8N + g*N + f*16 + r
        mw = np.ascontiguousarray(
            m.reshape(n_calls, 8, F, 16, 2).transpose(0, 1, 3, 2, 4)
            .reshape(n_calls, 128, F, 2))
        # r-major layout: [call, 16g+r, f, d] <- pixel call*8N + g*N + r*F + f
        mr = np.ascontiguousarray(
            m.reshape(n_calls, 128, F, 2))
        in_maps.append({
            "source": src,
            "motions_w": mw,
            "motions_r": mr,
            "sel0": sel0,
            "sel1": sel1,
            "ident": ident,
        })
    return in_maps


def unscramble(out_dev, bs=16, K=11, h=256, w=256):
    """out_dev: [n_cores, n_calls, 128m, nch, 4B, 24(g,c)] -> (bs,K,h,w,3)."""
    n_cores, n_calls, _, nch, nB, _ = out_dev.shape
    o = out_dev.reshape(n_cores, n_calls, 128, nch, nB, 8, 3)
    # pixel n = call*8N + g*N + ch*512 + B*128 + m
    o = o.transpose(0, 1, 5, 3, 4, 2, 6)   # cores, call, g, ch, B, m, c
    return o.astype(np.float32).reshape(bs, K, h, w, 3)


def kernel(source, motions):
    from concourse import bass_utils

    bs, K, h, w, _ = motions.shape          # (16, 11, 256, 256, 2)
    n_cores = 8
    grids = bs * K                          # 176
    per_core = grids // n_cores             # 22
    n_pix = per_core * h * w                # 1441792
    N = 4096

    nc = _get_compiled(n_pix, N)
    in_maps = prepare_in_maps(source, motions, n_cores=n_cores, N=N)
    res = bass_utils.run_bass_kernel_spmd(nc, in_maps, core_ids=list(range(n_cores)))
    out = np.stack([res.results[i]["out"] for i in range(n_cores)], axis=0)
    return unscramble(out, bs, K, h, w)



# revision 16
# speedup vs baseline: 1.5152x; 1.0232x over previous
"""Self-contained Trainium2 Bass kernel for nn_Deform (bilinear grid sample).

kernel(source, motions) -> (16, 11, 256, 256, 3) float32
Shards the 176 (bs*K) grids across 8 NeuronCores (22 grids each); the
source image is replicated. See build_kernel docstring for the device
algorithm.
"""

"""Bilinear grid-sample (Deform) Bass kernel for TRN2.

Strategy:
- 12 fp16 "pair planes" per 16-partition group: plane j = (c, dy, v) holds
  pairs (img[y+dy, 2q+v], img[y+dy, 2q+v+1]) at pair index i = y*128 + q,
  zero outside the image. All 8 Q7 groups hold identical plane sets.
- One ap_gather index per pixel: i0 = clamp(yn,0,255)*128 + clamp(xw,0,255)>>1.
  Gather returns, for pixel slot m of group g, the 12 planes' pairs.
- Per-pixel blend coefficients ACE[dy,vv,e] = wy_dy*masky_dy * wx_e*maskx_e *
  (parity==vv), computed on DVE in "L1" layout, moved to gather-aligned rows
  by strided SBUF->SBUF DMAs, broadcast to the 12 plane rows by a constant
  selector matmul on PE.
- out_c = sum_{dy,vv,e} ACE * G[(c,dy,vv), m, e]; reduced via PE transpose +
  DVE strided segment reduce; one DMA per 512-pixel chunk to HBM.

Pixel id n (per core) = call*(8*N) + g*N + m, m = f*16 + r.
"""

import numpy as np

import concourse.bass as bass
import concourse.mybir as mybir

F32 = mybir.dt.float32
F16 = mybir.dt.float16
I16 = mybir.dt.int16

H = W = 256
C = 3
NE = 32768          # pairs per plane (= 256*128)
NPLANE = 12         # (c, dy, v)


def plane_id(c, dy, v):
    return c * 4 + dy * 2 + v


def make_selector(e):
    """S[k, mcol] fp16: CW_e[16g + j] <- QR row 16g + t(j), t = dy*4 + v*2 + e."""
    S = np.zeros((128, 128), np.float16)
    for g in range(8):
        for j in range(NPLANE):
            c, rem = divmod(j, 4)
            dy, v = divmod(rem, 2)
            t = dy * 4 + v * 2 + e
            S[16 * g + t, 16 * g + j] = 1.0
    return S


def make_identity128():
    return np.eye(128, dtype=np.float16)


def make_sel24():
    """S[16g+j, 3g+c] = 1 for j in [4c, 4c+4): sums the 4 (dy,v) products."""
    S = np.zeros((128, 24), np.float16)
    for g in range(8):
        for c in range(3):
            for s in range(4):
                S[16 * g + 4 * c + s, 3 * g + c] = 1.0
    return S


def build_kernel(nc, n_pix, N=4096, dbg=False, skip_gather=False, only_gather=False):
    """Emit the full kernel into `nc`. n_pix must be a multiple of 8*N."""
    from concourse.tile import TileContext

    CALL_PIX = 8 * N
    assert n_pix % CALL_PIX == 0
    n_calls = n_pix // CALL_PIX
    F = N // 16          # slots per partition-lane
    NQ = 8               # quantities (coefficients) per pixel
    CH = 512             # blend chunk (pixels-slots per group)
    n_chunks = N // CH

    src = nc.dram_tensor("source", [1, H, W, C], F32, kind="ExternalInput")
    # pre-permuted on host to the exact SBUF tile layouts (fat DMAs)
    mot_w = nc.dram_tensor("motions_w", [n_calls, 128, N // 16, 2], F32,
                           kind="ExternalInput")
    mot_r = nc.dram_tensor("motions_r", [n_calls, 128, N // 16, 2], F32,
                           kind="ExternalInput")
    sel0_d = nc.dram_tensor("sel0", [128, 128], F16, kind="ExternalInput")
    sel1_d = nc.dram_tensor("sel1", [128, 128], F16, kind="ExternalInput")
    ident_d = nc.dram_tensor("ident", [128, 128], F16, kind="ExternalInput")
    sel24_d = nc.dram_tensor("sel24", [128, 24], F16, kind="ExternalInput")
    # device-layout output; host unscrambles
    out = nc.dram_tensor("out", [n_calls, 128, N // CH, 4, 24], F16,
                         kind="ExternalOutput")

    with TileContext(nc) as tc:
        with (
            tc.tile_pool(name="pln", bufs=1) as plnp,
            tc.tile_pool(name="const", bufs=1) as constp,
            tc.tile_pool(name="work", bufs=1) as workp,
            tc.tile_pool(name="pipe", bufs=2) as pipep,
            tc.tile_pool(name="psum", bufs=2, space="PSUM") as psump,
            tc.tile_pool(name="dram", bufs=1, space="DRAM") as dramp,
        ):
            # ---- constants ----
            sel0 = constp.tile([128, 128], F16)
            sel1 = constp.tile([128, 128], F16)
            ident = constp.tile([128, 128], F16)
            sel24 = constp.tile([128, 24], F16)
            nc.sync.dma_start(sel24[:], sel24_d[:])
            nc.sync.dma_start(sel0[:], sel0_d[:])
            nc.sync.dma_start(sel1[:], sel1_d[:])
            nc.sync.dma_start(ident[:], ident_d[:])

            # ---- plane table: [128, NE, 2] fp16 ----
            # stage fp16 channel images (+pad) in DRAM, then planes are
            # contiguous shifted reads (edge garbage is weight-masked).
            PAD = 512
            ch_d = dramp.tile([3, H * W + PAD], F16)
            s0 = pipep.tile([128, 512, 3], F32, tag="gth")
            s16 = pipep.tile([128, 3, 512], F16, tag="qr16")
            zz = pipep.tile([128, 4], F16, tag="idx")
            nc.sync.dma_start(
                s0[:], src[0].rearrange("h w c -> (h w) c").rearrange(
                    "(p k) c -> p k c", p=128))
            nc.vector.tensor_copy(s16[:], s0[:].rearrange("p k c -> p c k"))
            nc.vector.memset(zz[:], 0.0)
            for c in range(C):
                nc.sync.dma_start(
                    ch_d[c, :H * W].rearrange("(p k) -> p k", p=128),
                    s16[:, c, :])
            nc.sync.dma_start(
                ch_d[:, H * W:].rearrange("c (p q) -> p c q", p=128),
                zz[:128, :].rearrange("p (c q) -> p c q", c=1).to_broadcast([128, 3, 4]))
            pln = plnp.tile([128, NE, 2], F16)
            nc.vector.memset(pln[:, :NE // 2, :], 0.0)
            nc.vector.memset(pln[:, NE // 2:, :], 0.0)
            flat = pln[:].rearrange("p n d -> p (n d)")  # [128, 65536]
            for g in range(8):
                for c in range(C):
                    for dy in range(2):
                        for v in range(2):
                            p = 16 * g + plane_id(c, dy, v)
                            off = dy * W + v
                            nc.sync.dma_start(
                                flat[p:p + 1, :],
                                ch_d[c, off:off + H * W],
                            )

            # zero qr16 pool slots once: rows 16g+8..16 are never written
            for _ in range(2):
                qz = pipep.tile([128, N], F16, tag="qr16")
                nc.vector.memset(qz[:], 0.0)

            # ---- main loop ----
            for call in range(n_calls):
                # motions tile (wrap layout): partition 16g+r <- pixel base + g*N + f*16 + r
                motv = pipep.tile([128, F, 2], F32, tag="motv")
                # motions tile (r-major): partition 16g+r <- pixels [g*N + r*F, +F)
                motq = pipep.tile([128, F, 2], F32, tag="motq")
                nc.sync.dma_start(motv[:], mot_w[call])
                nc.sync.dma_start(motq[:], mot_r[call])
                gx = motv[:, :, 0]
                gy = motv[:, :, 1]
                qx = motq[:, :, 0]
                qy = motq[:, :, 1]

                # ---- phase A: per-pixel quantities (f32, L1 layout) ----
                q8 = workp.tile([128, NQ, F], F16, tag="q8")
                idx = pipep.tile([128, F], I16, tag="idx")
                t1 = workp.tile([128, F], F32, tag="t1")
                t2 = workp.tile([128, F], F32, tag="t2")
                txw = workp.tile([128, F], F32, tag="txw")
                tyw = workp.tile([128, F], F32, tag="tyw")
                fx = workp.tile([128, F], F32, tag="fx")
                fy = workp.tile([128, F], F32, tag="fy")
                b0 = workp.tile([128, F], F32, tag="b0")
                b1 = workp.tile([128, F], F32, tag="b1")
                a0 = workp.tile([128, F], F32, tag="a0")
                a1 = workp.tile([128, F], F32, tag="a1")
                vpar = tyw   # alias: tyw dead before parity computation
                i0f = fx     # alias: fx dead before i0 computation

                AL = mybir.AluOpType
                # ---- idx path (wrap layout) ----
                nc.vector.tensor_scalar(t1[:], gx, 128.0, 143.5, AL.mult, AL.add)
                nc.vector.tensor_scalar(t2[:], t1[:], 12582912.0, 12582912.0, AL.add, AL.subtract)
                nc.vector.tensor_tensor(fy[:], t2[:], t1[:], AL.is_gt)
                nc.vector.tensor_tensor(txw[:], t2[:], fy[:], AL.subtract)
                nc.vector.tensor_scalar(t1[:], gy, 128.0, 143.5, AL.mult, AL.add)
                nc.vector.tensor_scalar(t2[:], t1[:], 12582912.0, 12582912.0, AL.add, AL.subtract)
                nc.vector.tensor_tensor(fy[:], t2[:], t1[:], AL.is_gt)
                nc.vector.tensor_tensor(tyw[:], t2[:], fy[:], AL.subtract)
                nc.vector.tensor_scalar(t1[:], txw[:], 16.0, 271.0, AL.max, AL.min)
                nc.vector.tensor_scalar(t1[:], t1[:], 0.5, -8.0, AL.mult, AL.add)
                nc.vector.tensor_scalar(fy[:], t1[:], 12582912.0, 12582912.0, AL.add, AL.subtract)
                nc.vector.tensor_tensor(b0[:], fy[:], t1[:], AL.is_gt)
                nc.vector.tensor_tensor(t2[:], fy[:], b0[:], AL.subtract)
                nc.vector.tensor_scalar(t1[:], tyw[:], 16.0, 271.0, AL.max, AL.min)
                nc.vector.tensor_scalar(t2[:], t2[:], -2048.0, None, AL.add)
                nc.vector.scalar_tensor_tensor(
                    i0f[:], t1[:], 128.0, t2[:], AL.mult, AL.add)
                nc.vector.tensor_copy(idx[:], i0f[:])

                # ---- coefficient path (r-major layout) ----
                nc.vector.tensor_scalar(t1[:], qx, 128.0, 143.5, AL.mult, AL.add)
                nc.vector.tensor_scalar(t2[:], t1[:], 12582912.0, 12582912.0, AL.add, AL.subtract)
                nc.vector.tensor_tensor(fx[:], t2[:], t1[:], AL.is_gt)
                nc.vector.tensor_tensor(txw[:], t2[:], fx[:], AL.subtract)
                nc.vector.tensor_tensor(fx[:], t1[:], txw[:], AL.subtract)
                nc.vector.tensor_scalar(t1[:], qy, 128.0, 143.5, AL.mult, AL.add)
                nc.vector.tensor_scalar(t2[:], t1[:], 12582912.0, 12582912.0, AL.add, AL.subtract)
                nc.vector.tensor_tensor(fy[:], t2[:], t1[:], AL.is_gt)
                nc.vector.tensor_tensor(tyw[:], t2[:], fy[:], AL.subtract)
                nc.vector.tensor_tensor(fy[:], t1[:], tyw[:], AL.subtract)
                # x masks -> B0/B1
                nc.vector.tensor_scalar(t1[:], txw[:], 15.0, None, AL.is_gt)
                nc.vector.tensor_scalar(t2[:], txw[:], 272.0, None, AL.is_lt)
                nc.vector.tensor_tensor(b0[:], t1[:], t2[:], AL.mult)
                nc.vector.tensor_scalar(t1[:], txw[:], 14.0, None, AL.is_gt)
                nc.vector.tensor_scalar(t2[:], txw[:], 271.0, None, AL.is_lt)
                nc.vector.tensor_tensor(b1[:], t1[:], t2[:], AL.mult)
                nc.vector.tensor_scalar(t1[:], fx[:], -1.0, 1.0, AL.mult, AL.add)
                nc.vector.tensor_tensor(b0[:], b0[:], t1[:], AL.mult)
                nc.vector.tensor_tensor(b1[:], b1[:], fx[:], AL.mult)
                # xw == -1: e-corner lands in pair elem 0 -> swap coefficients
                nc.vector.tensor_scalar(t1[:], txw[:], 15.0, None, AL.is_equal)
                nc.vector.tensor_tensor(t2[:], t1[:], b1[:], AL.mult)
                nc.vector.tensor_tensor(b0[:], b0[:], t2[:], AL.add)
                nc.vector.tensor_scalar(t1[:], t1[:], -1.0, 1.0, AL.mult, AL.add)
                nc.vector.tensor_tensor(b1[:], b1[:], t1[:], AL.mult)
                # y masks -> A0/A1
                nc.vector.tensor_scalar(t1[:], tyw[:], 15.0, None, AL.is_gt)
                nc.vector.tensor_scalar(t2[:], tyw[:], 272.0, None, AL.is_lt)
                nc.vector.tensor_tensor(a0[:], t1[:], t2[:], AL.mult)
                nc.vector.tensor_scalar(t1[:], tyw[:], 14.0, None, AL.is_gt)
                nc.vector.tensor_scalar(t2[:], tyw[:], 271.0, None, AL.is_lt)
                nc.vector.tensor_tensor(a1[:], t1[:], t2[:], AL.mult)
                nc.vector.tensor_scalar(t1[:], fy[:], -1.0, 1.0, AL.mult, AL.add)
                nc.vector.tensor_tensor(a0[:], a0[:], t1[:], AL.mult)
                nc.vector.tensor_tensor(a1[:], a1[:], fy[:], AL.mult)
                # yn == -1: s-corner lands in dy=0 plane -> swap coefficients
                nc.vector.tensor_scalar(t1[:], tyw[:], 15.0, None, AL.is_equal)
                nc.vector.tensor_tensor(t2[:], t1[:], a1[:], AL.mult)
                nc.vector.tensor_tensor(a0[:], a0[:], t2[:], AL.add)
                nc.vector.tensor_scalar(t1[:], t1[:], -1.0, 1.0, AL.mult, AL.add)
                nc.vector.tensor_tensor(a1[:], a1[:], t1[:], AL.mult)
                # parity v
                nc.vector.tensor_scalar(t1[:], txw[:], 16.0, 271.0, AL.max, AL.min)
                nc.vector.tensor_scalar(t1[:], t1[:], 0.5, -8.0, AL.mult, AL.add)
                nc.vector.tensor_scalar(t2[:], t1[:], 12582912.0, 12582912.0, AL.add, AL.subtract)
                nc.vector.tensor_tensor(fx[:], t2[:], t1[:], AL.is_gt)
                nc.vector.tensor_tensor(vpar[:], t2[:], fx[:], AL.subtract)
                nc.vector.tensor_tensor(vpar[:], t1[:], vpar[:], AL.subtract)
                nc.vector.tensor_scalar(vpar[:], vpar[:], 2.0, None, AL.mult)
                # ACE quantities t = dy*4 + vv*2 + e
                nc.vector.tensor_scalar(t1[:], vpar[:], -1.0, 1.0, AL.mult, AL.add)
                for dy, at in ((0, a0), (1, a1)):
                    for e, bt in ((0, b0), (1, b1)):
                        nc.vector.tensor_tensor(t2[:], at[:], bt[:], AL.mult)
                        nc.vector.tensor_tensor(
                            q8[:, dy * 4 + 0 * 2 + e, :], t2[:], t1[:], AL.mult)
                        nc.vector.tensor_tensor(
                            q8[:, dy * 4 + 1 * 2 + e, :], t2[:], vpar[:], AL.mult)

                # ---- phase C: unwrap DMAs -> QR [128, N] f32 rows 16g+t ----
                qr16 = pipep.tile([128, N], F16, tag="qr16")
                qstage = dramp.tile([8, NQ, N], F16, tag="qstage")
                for g in range(8):
                    nc.sync.dma_start(
                        qstage[g].rearrange("t (r f) -> r t f", r=16),
                        q8[16 * g:16 * g + 16, :, :],
                    )
                for g in range(8):
                    nc.sync.dma_start(
                        qr16[16 * g:16 * g + NQ, :],
                        qstage[g],
                    )

                # ---- phase D: gather ----
                gth = pipep.tile([128, N, 2], F16, tag="gth")
                if skip_gather:
                    pass
                else:
                        nc.gpsimd.ap_gather(
                        out_ap=gth[:],
                        in_ap=pln[:],
                        idxs_ap=idx[:],
                        channels=128,
                        num_elems=NE,
                        d=2,
                        num_idxs=N,
                    )
                if only_gather:
                    continue

                # ---- phase E/F: blend per chunk ----
                # g_e = gth[:,:,e] * (sel_e @ qr16); ps24[(g,c)] = sel24-sum of
                # both; transpose back to pixel-partition layout; ACT copies.
                og = pipep.tile([128, n_chunks, 4, 24], F16, tag="og")
                CP = mybir.ActivationFunctionType.Copy
                for ch in range(n_chunks):
                    sl = slice(ch * CH, (ch + 1) * CH)
                    cw0 = psump.tile([128, CH], F32, tag="cw0")
                    cw1 = psump.tile([128, CH], F32, tag="cw1")
                    nc.tensor.matmul(cw0[:], sel0[:], qr16[:, sl], start=True, stop=True)
                    nc.tensor.matmul(cw1[:], sel1[:], qr16[:, sl], start=True, stop=True)
                    g0 = workp.tile([128, CH], F16, tag="g0")
                    g1 = workp.tile([128, CH], F16, tag="g1")
                    nc.vector.tensor_tensor(g0[:], gth[:, sl, 0], cw0[:], AL.mult)
                    nc.vector.tensor_tensor(g1[:], gth[:, sl, 1], cw1[:], AL.mult)
                    ps24 = psump.tile([24, CH], F32, tag="ps24")
                    nc.tensor.matmul(ps24[:], sel24[:, 0:24], g0[:], start=True, stop=False)
                    nc.tensor.matmul(ps24[:], sel24[:, 0:24], g1[:], start=False, stop=True)
                    sb24 = workp.tile([24, CH], F16, tag="sb24")
                    nc.scalar.activation(sb24[:], ps24[:], CP)
                    tp4 = psump.tile([128, 4, 24], F16, tag="tp4")
                    for B in range(4):
                        nc.tensor.transpose(
                            tp4[:, B, :],
                            sb24[:, B * 128:(B + 1) * 128],
                            ident[0:24, 0:24],
                        )
                    nc.scalar.activation(og[:, ch, :, :], tp4[:], CP)
                # out DMA: device-layout, one fat contiguous DMA per call
                nc.sync.dma_start(out[call], og[:])
    return nc


_COMPILED = {}


def _get_compiled(n_pix, N):
    key = (n_pix, N)
    if key not in _COMPILED:
        import concourse.bacc as bacc
        nc = bacc.Bacc("TRN2", target_bir_lowering=False, debug=False)
        build_kernel(nc, n_pix, N=N)
        nc.finalize()
        _COMPILED[key] = nc
    return _COMPILED[key]


def prepare_in_maps(source, motions, n_cores=8, N=4096):
    """Host-side permutation into the exact device tile layouts."""
    bs, K, h, w, _ = motions.shape          # (16, 11, 256, 256, 2)
    grids = bs * K                          # 176
    n_pix = (grids // n_cores) * h * w      # 1441792 per core
    n_calls = n_pix // (8 * N)              # 44
    F = N // 16                             # 256

    mot_flat = motions.reshape(n_cores, n_pix, 2)
    src = np.ascontiguousarray(source, dtype=np.float32)
    sel0 = make_selector(0)
    sel1 = make_selector(1)
    ident = make_identity128()

    in_maps = []
    for i in range(n_cores):
        m = mot_flat[i]
        # wrap layout: [call, 16g+r, f, d] <- pixel call*8N + g*N + f*16 + r
        mw = np.ascontiguousarray(
            m.reshape(n_calls, 8, F, 16, 2).transpose(0, 1, 3, 2, 4)
            .reshape(n_calls, 128, F, 2))
        # r-major layout: [call, 16g+r, f, d] <- pixel call*8N + g*N + r*F + f
        mr = np.ascontiguousarray(
            m.reshape(n_calls, 128, F, 2))
        in_maps.append({
            "source": src,
            "motions_w": mw,
            "motions_r": mr,
            "sel0": sel0,
            "sel1": sel1,
            "ident": ident,
            "sel24": make_sel24(),
        })
    return in_maps


def unscramble(out_dev, bs=16, K=11, h=256, w=256):
    """out_dev: [n_cores, n_calls, 128m, nch, 4B, 24(g,c)] -> (bs,K,h,w,3)."""
    n_cores, n_calls, _, nch, nB, _ = out_dev.shape
    o = out_dev.reshape(n_cores, n_calls, 128, nch, nB, 8, 3)
    # pixel n = call*8N + g*N + ch*512 + B*128 + m
    o = o.transpose(0, 1, 5, 3, 4, 2, 6)   # cores, call, g, ch, B, m, c
    return o.astype(np.float32).reshape(bs, K, h, w, 3)


def kernel(source, motions):
    from concourse import bass_utils

    bs, K, h, w, _ = motions.shape          # (16, 11, 256, 256, 2)
    n_cores = 8
    grids = bs * K                          # 176
    per_core = grids // n_cores             # 22
    n_pix = per_core * h * w                # 1441792
    N = 4096

    nc = _get_compiled(n_pix, N)
    in_maps = prepare_in_maps(source, motions, n_cores=n_cores, N=N)
    res = bass_utils.run_bass_kernel_spmd(nc, in_maps, core_ids=list(range(n_cores)))
    out = np.stack([res.results[i]["out"] for i in range(n_cores)], axis=0)
    return unscramble(out, bs, K, h, w)

